# revision 21
# baseline (speedup 1.0000x reference)
"""Gaussian NLL loss kernel for Trainium2 (8 NeuronCores, data-parallel).

out[n] = 0.5 * (x_n - mu)^T pinv(sigma+eps) (x_n - mu) + log_den,  shape [N, 1]

Strategy (v4, fp8 e3m4):
  Host: tiny D x D prep (pinv -> symmetrize -> Cholesky L, slogdet); center
  X by mu and cast to fp8 E3M4 (quarters HBM traffic vs fp32; 4 mantissa
  bits keep rel err ~6e-3 << 2e-2 budget); one-time transpose so each
  core's DMA loads are contiguous per partition (features on partitions).
  Device (per core, N/8 samples, software-pipelined):
    z    = (L/sqrt 2)^T x'         mixed fp16(L) x fp8(x) matmul
    zsq  = z^2                     split ~84/44 on ScalarE Square (fused
                                   PSUM-drain+square) and DVE (fp32->fp16
                                   copy, then fp16 2x mul).  The PSUM
                                   drain (ACT@1.2GHz + DVE@0.96GHz per
                                   lane) is the fundamental bottleneck.
    q    = colsum(zsq)             one-hot selector matmuls, 4 CONCURRENT
                                   col-tiled streams (tile_position
                                   32*cg); batched 16 MMs per half-chunk
                                   so the PE pays the stationary-weight
                                   switch (~170ns drain) only 32x total
    out  = q + log_den             one [128,512] DVE add per chunk
  Pure data-parallel: no collectives.
"""

import math
import sys

import numpy as np

sys.path.insert(0, "/opt/trn_rl_repo")

import ml_dtypes

import concourse.bass as bass
import concourse.bacc as bacc
import concourse.mybir as mybir
import concourse.tile as tile
from concourse.bass_utils import run_bass_kernel_spmd

N, D = 1048576, 128
NCORES = 8
NSH = N // NCORES   # 131072 samples per core
CHUNK = 16384       # samples per x tile (2MB fp8), loaded as 4 sub-DMAs
SUBDMA = 4096       # columns per input DMA (512KB)
GROUP = 1024        # samples per square op (2 PSUM banks)
SUB = 512           # samples per matmul (PSUM bank limit)
GPC = CHUNK // GROUP   # 16 groups per chunk
SPC = CHUNK // SUB     # 32 colsum sub-chunks per chunk
HC = GPC // 2          # groups per half-chunk (sel batch unit)
NCHUNK = NSH // CHUNK  # 8 chunks per core
NG = NSH // GROUP      # 128 groups per core
# which groups in a chunk square on DVE (rest on ScalarE); ACT ~1103ns vs
# DVE copy+mul ~1912ns per group -> 83:45 split over the 128 groups.
# Groups 0-3 of each chunk MUST be on ACT: with the single start=True on
# sub 0, in-order ACT completion + the pz-ring WAR chain guarantee sub 0's
# zsq is ready before any other sub's, so the bank-clearing matmul is
# scheduled first (anything else is a correctness race).
DVE_5 = (4, 6, 9, 11, 14)
DVE_6 = (4, 6, 8, 10, 12, 14)
DVE_PATTERN = (DVE_6, DVE_6, DVE_5, DVE_6, DVE_6, DVE_5, DVE_6, DVE_5)

_f32 = mybir.dt.float32
_f16 = mybir.dt.float16
_f8 = mybir.dt.float8e3

LAST_RESULTS = None  # BassKernelResults of the most recent run (for test.py)


def _build_bass(log_den: float) -> bass.Bass:
    nc = bacc.Bacc()
    xt = nc.declare_dram_parameter("xt", [D, NSH], _f8, isOutput=False)
    # fp16 consts: cols 0-127 = L/sqrt2; cols 128+8j..+8 = one-hot sel mat
    # j; col 192 = scratch the sel-batch gate writes into
    wc = nc.declare_dram_parameter("wc", [D, D + 65], _f16, isOutput=False)
    out = nc.declare_dram_parameter("out", [1, NSH], _f32, isOutput=True)

    with tile.TileContext(nc) as tc:
        with (
            tc.tile_pool(name="const", bufs=1) as cpool,
            tc.tile_pool(name="xin", bufs=12) as xpool,
            tc.tile_pool(name="zsq", bufs=2 * HC + 4) as zpool,
            tc.tile_pool(name="zc", bufs=2) as zcpool,
            tc.tile_pool(name="outs", bufs=2) as opool,
            tc.tile_pool(name="gate", bufs=3) as gpool,
            tc.tile_pool(name="pz", bufs=3, space=bass.MemorySpace.PSUM) as pzpool,
            tc.tile_pool(name="pq", bufs=2, space=bass.MemorySpace.PSUM) as pqpool,
        ):
            wc_t = cpool.tile([D, D + 65], _f16, name="wc_t")
            nc.sync.dma_start(out=wc_t[:], in_=wc[:])
            lw_t = wc_t[:, 0:D]
            sel_t = wc_t[:, D:D + 64]
            # Pre-sync the PE on the const DMA with one dummy matmul so real
            # matmuls never stall on it; also starts the HAM warm-up clock.
            warm = pqpool.tile([8, 8], _f32, tag="pq", name="warm")
            nc.tensor.matmul(warm[:], sel_t[:, 0:8], sel_t[:, 0:8])
            # Pre-warm the ACT square table (~2.7us) during the initial DMA.
            aw2 = cpool.tile([D, 1], _f16, name="aw2")
            nc.scalar.activation(
                aw2[:], sel_t[:, 0:1], mybir.ActivationFunctionType.Square
            )
            # zero rhs for the accumulator-bank clearing matmul
            zrhs = cpool.tile([D, SUB], _f16, name="zrhs")
            nc.vector.memset(zrhs[:], 0.0)

            x_tiles = {}   # chunk -> sbuf tile
            pq_tiles = {}  # chunk -> psum accum tile
            zq_tiles = {}  # group -> sbuf tile

            def load_chunk(c, splits=None):
                # separate sub-tiles per DMA so z matmuls depend only on
                # the slice they read (fast ramp, fine-grained prefetch)
                x_tiles[c] = []
                lo = 0
                for w in (splits or [SUBDMA] * (CHUNK // SUBDMA)):
                    x_q = xpool.tile([D, SUBDMA], _f8, tag="x", name="x_q")
                    nc.sync.dma_start(
                        out=x_q[:, 0:w],
                        in_=xt[:, c * CHUNK + lo:c * CHUNK + lo + w],
                    )
                    x_tiles[c].append((lo, w, x_q))
                    lo += w

            def z_and_square(g):
                c = g // GPC
                if g == 0:
                    # chunk 0 split finer up front so group 0's data lands asap
                    load_chunk(0, splits=[GROUP, SUBDMA - GROUP]
                               + [SUBDMA] * (CHUNK // SUBDMA - 1))
                if g % GPC == 0:
                    pq_tiles[c] = pqpool.tile(
                        [D, SUB], _f32, tag="pq", name="pq"
                    )
                if g % GPC == HC and c + 1 < NCHUNK:
                    load_chunk(c + 1)  # prefetch ~11us ahead of first use
                pz = pzpool.tile([D, GROUP], _f32, tag="pz", name="pz")
                gbase = (g % GPC) * GROUP
                for s in range(GROUP // SUB):
                    lo = gbase + s * SUB
                    # find the sub-tile holding columns [lo, lo+SUB)
                    for tlo, w, x_q in x_tiles[c]:
                        if tlo <= lo < tlo + w:
                            break
                    nc.tensor.matmul(
                        pz[:, s * SUB:(s + 1) * SUB],
                        lw_t,
                        x_q[:, lo - tlo:lo - tlo + SUB],
                    )
                zq = zpool.tile([D, GROUP], _f16, tag="zq", name="zq")
                dve = (g % GPC) in DVE_PATTERN[c % len(DVE_PATTERN)]
                if dve:
                    # Square on the DVE. DVE reads only ONE operand from
                    # PSUM, so detour: copy pz -> fp16 SBUF (1x), then fp16
                    # tensor_tensor square (2x).
                    zc = zcpool.tile([D, GROUP], _f16, tag="zc", name="zc")
                    nc.vector.tensor_copy(zc[:], pz[:])
                    nc.vector.tensor_mul(zq[:], zc[:], zc[:])
                else:
                    nc.scalar.activation(
                        zq[:], pz[:], mybir.ActivationFunctionType.Square
                    )
                zq_tiles[g] = zq  # consumed one half-chunk later

            def emit_sels(c, subs, gate_zq=None):
                """Colsum matmuls for sub-chunks ``subs`` of chunk c, as one
                batch: the PE pays the L<->sel weight switch only twice.

                gate_zq: if given, the sels read a GpSimd-made copy of the
                selector weights whose FIFO predecessor reads gate_zq — so
                the whole batch becomes schedulable only after gate_zq is
                written.  Without the gate, Tile's priority scheduler
                dribbles the sels into every z-matmul stall in ~3-MM runs,
                paying the L<->sel weight-switch (~200ns) ~85x instead of
                ~32x."""
                if gate_zq is not None:
                    # data-chain so the batch is unschedulable until gate_zq
                    # lands: zq slice -> wc_t col 192, then live copies cols
                    # 128..193 (includes 192) and the sels read live.
                    nc.gpsimd.tensor_copy(wc_t[:, D + 64:D + 65],
                                          gate_zq[:, 0:1])
                    live = gpool.tile([D, 65], _f16, tag="live", name="live")
                    nc.gpsimd.tensor_copy(live[:], wc_t[:, D:D + 65])
                else:
                    live = sel_t
                if subs[0] == 0:
                    # Clear the whole accumulator bank with one L-weight x
                    # zero-rhs matmul (out = 0): start=True resets the
                    # bank's has_written state exactly ONCE per chunk, over
                    # ALL partitions; every sel below overlaps it so WAW
                    # deps order it first, and the remaining accumulation
                    # is commutative — correct under any scheduling.  One
                    # start per col-group chain instead is a RACE (observed
                    # nondeterministic wrong results).  L-as-weights keeps
                    # the stationary switch count unchanged; emitting here
                    # (not at chunk start) keeps the in-order PE queue from
                    # blocking at runtime on the c-2 accumulator drain.
                    nc.tensor.matmul(
                        pq_tiles[c][:, :],
                        lw_t,
                        zrhs[:],
                        start=True,
                        stop=False,
                        skip_group_check=True,
                    )
                for s in subs:
                    g = c * GPC + s // 2
                    cg, j = s % 4, s // 4
                    zqq = zq_tiles[g] if s % 2 == 0 else zq_tiles.pop(g)
                    # col-tiled: psum slice base partition 32*cg selects the
                    # array col-group; the 4 cg streams run concurrently.
                    nc.tensor.matmul(
                        pq_tiles[c][32 * cg:32 * cg + 8, :],
                        live[:, 8 * j:8 * j + 8],
                        zqq[:, (s % 2) * SUB:(s % 2 + 1) * SUB],
                        start=False,
                        stop=(s >= SPC - 4),
                        skip_group_check=True,
                        tile_position=(0, 32 * cg),
                    )
                if subs[-1] == SPC - 1:
                    o_t = opool.tile([D, SUB], _f32, tag="o", name="o_t")
                    # +log_den on DVE over the whole [128, 512] accum tile
                    # (only partitions 32*cg + j carry data; rest is junk)
                    nc.vector.tensor_scalar_add(
                        o_t[:], pq_tiles.pop(c)[:], float(log_den)
                    )
                    # sample index within chunk = s*SUB + n, s = 4*j + cg
                    oview = out[:, c * CHUNK:(c + 1) * CHUNK].rearrange(
                        "a (j r) -> (a j) r", j=8
                    )
                    for cg in range(4):
                        nc.sync.dma_start(
                            out=oview[:, cg * SUB:(cg + 1) * SUB],
                            in_=o_t[32 * cg:32 * cg + 8, :],
                        )

            # Software pipeline: z+square stream ahead group by group; the
            # batched sels for each half-chunk run one half-chunk behind,
            # gated so each batch hits the PE as one contiguous burst.
            HALF = list(range(0, 16)), list(range(16, 32))
            for g in range(NG):
                z_and_square(g)
                if g % HC == HC - 1 and g >= 2 * HC - 1:
                    hc = g // HC - 1
                    emit_sels(hc // 2, HALF[hc % 2])
            emit_sels(NCHUNK - 1, HALF[1])  # epilogue, ungated
    nc.compile()
    return nc


def _install_trace_shim():
    """The image lacks ``antenv.axon_hooks``; recreate it and register the
    ctypes NTFF hook that trn_boot would have installed."""
    import types
    import antenv

    if "antenv.axon_hooks" not in sys.modules:
        mod = types.ModuleType("antenv.axon_hooks")
        holder = [None]
        mod.set_axon_ntff_profile_hook = lambda h: holder.__setitem__(0, h)
        mod.get_axon_ntff_profile_hook = lambda: holder[0]
        sys.modules["antenv.axon_hooks"] = mod
        antenv.axon_hooks = mod
    from antenv.axon_hooks import (
        get_axon_ntff_profile_hook,
        set_axon_ntff_profile_hook,
    )

    if get_axon_ntff_profile_hook() is None:
        from trn_agent_boot.trn_boot import _ntff_profile_via_ctypes

        set_axon_ntff_profile_hook(
            _ntff_profile_via_ctypes("/opt/axon/libaxon_pjrt.so")
        )


def kernel(X: np.ndarray, mu: np.ndarray, sigma: np.ndarray, eps: np.ndarray,
           _trace: bool = False) -> np.ndarray:
    global LAST_RESULTS

    # ---- host prep: tiny D x D linear algebra in float64 ----
    sig = (sigma.astype(np.float64) + eps.astype(np.float64))
    S = np.linalg.pinv(sig)
    _, logdet = np.linalg.slogdet(sig)
    log_den = 0.5 * (D * math.log(2.0 * math.pi) + logdet)
    Ssym = 0.5 * (S + S.T)
    L = np.linalg.cholesky(Ssym)          # S = L @ L.T
    Lp = (L / math.sqrt(2.0)).astype(np.float16)  # [d, e] stationary

    wcat = np.zeros((D, D + 65), dtype=np.float16)
    wcat[:, :D] = Lp
    for j in range(8):
        wcat[:, D + 8 * j + j] = 1.0

    # ---- center by mu on host (removes the device-side bias), cast fp8 ----
    f8 = ml_dtypes.float8_e3m4            # TRN FP8_EXP3: max +-15.5
    Xc = np.clip(X - mu[None, :], -15.0, 15.0).astype(f8)
    XT = np.ascontiguousarray(Xc.T)  # [D, N]
    in_maps = []
    for c_id in range(NCORES):
        in_maps.append({
            "xt": np.ascontiguousarray(XT[:, c_id * NSH:(c_id + 1) * NSH]),
            "wc": wcat,
        })

    nc = _build_bass(log_den)
    if _trace:
        _install_trace_shim()
        import tempfile
        import concourse.bass_utils as _bu
        _bu.upload_artifacts = lambda d: "local://" + d  # no S3 in this container
        tmpdir = tempfile.mkdtemp(prefix="bass_trace_")
        print("trace dir:", tmpdir)
        res = run_bass_kernel_spmd(
            nc, in_maps, list(range(NCORES)), trace=True, tmpdir=tmpdir
        )
    else:
        res = run_bass_kernel_spmd(nc, in_maps, list(range(NCORES)))
    LAST_RESULTS = res

    out = np.empty((N, 1), dtype=np.float32)
    for c_id in range(NCORES):
        out[c_id * NSH:(c_id + 1) * NSH, 0] = res.results[c_id]["out"].reshape(-1)
    return out


# revision 22
# speedup vs baseline: 1.0159x; 1.0159x over previous
"""Gaussian NLL loss kernel for Trainium2 (8 NeuronCores, data-parallel).

out[n] = 0.5 * (x_n - mu)^T pinv(sigma+eps) (x_n - mu) + log_den,  shape [N, 1]

Strategy (v4, fp8 e3m4):
  Host: tiny D x D prep (pinv -> symmetrize -> Cholesky L, slogdet); center
  X by mu and cast to fp8 E3M4 (quarters HBM traffic vs fp32; 4 mantissa
  bits keep rel err ~6e-3 << 2e-2 budget); one-time transpose so each
  core's DMA loads are contiguous per partition (features on partitions).
  Device (per core, N/8 samples, software-pipelined):
    z    = (L/sqrt 2)^T x'         mixed fp16(L) x fp8(x) matmul
    zsq  = z^2                     split ~84/44 on ScalarE Square (fused
                                   PSUM-drain+square) and DVE (fp32->fp16
                                   copy, then fp16 2x mul).  The PSUM
                                   drain (ACT@1.2GHz + DVE@0.96GHz per
                                   lane) is the fundamental bottleneck.
    q    = colsum(zsq)             one-hot selector matmuls, 4 CONCURRENT
                                   col-tiled streams (tile_position
                                   32*cg); batched 16 MMs per half-chunk
                                   so the PE pays the stationary-weight
                                   switch (~170ns drain) only 32x total
    out  = q + log_den             one [128,512] DVE add per chunk
  Pure data-parallel: no collectives.
"""

import math
import sys

import numpy as np

sys.path.insert(0, "/opt/trn_rl_repo")

import ml_dtypes

import concourse.bass as bass
import concourse.bacc as bacc
import concourse.mybir as mybir
import concourse.tile as tile
from concourse.bass_utils import run_bass_kernel_spmd

N, D = 1048576, 128
NCORES = 8
NSH = N // NCORES   # 131072 samples per core
CHUNK = 16384       # samples per x tile (2MB fp8), loaded as 4 sub-DMAs
SUBDMA = 4096       # columns per input DMA (512KB)
GROUP = 1024        # samples per square op (2 PSUM banks)
SUB = 512           # samples per matmul (PSUM bank limit)
GPC = CHUNK // GROUP   # 16 groups per chunk
SPC = CHUNK // SUB     # 32 colsum sub-chunks per chunk
HC = GPC // 2          # groups per half-chunk (sel batch unit)
NCHUNK = NSH // CHUNK  # 8 chunks per core
NG = NSH // GROUP      # 128 groups per core
# which groups in a chunk square on DVE (rest on ScalarE); ACT ~1103ns vs
# DVE copy+mul ~1912ns per group -> 83:45 split over the 128 groups.
# Groups 0-3 of each chunk MUST be on ACT: with the single start=True on
# sub 0, in-order ACT completion + the pz-ring WAR chain guarantee sub 0's
# zsq is ready before any other sub's, so the bank-clearing matmul is
# scheduled first (anything else is a correctness race).
DVE_5 = (4, 6, 9, 11, 14)
DVE_6 = (4, 6, 8, 10, 12, 14)
DVE_PATTERN = (DVE_6, DVE_6, DVE_5, DVE_6, DVE_6, DVE_5, DVE_6, DVE_5)

_f32 = mybir.dt.float32
_f16 = mybir.dt.float16
_f8 = mybir.dt.float8e3

LAST_RESULTS = None  # BassKernelResults of the most recent run (for test.py)


def _build_bass(log_den: float) -> bass.Bass:
    nc = bacc.Bacc()
    xt = nc.declare_dram_parameter("xt", [D, NSH], _f8, isOutput=False)
    # fp16 consts: cols 0-127 = L/sqrt2; cols 128+8j..+8 = one-hot sel mat
    # j; col 192 = scratch the sel-batch gate writes into
    wc = nc.declare_dram_parameter("wc", [D, D + 65], _f16, isOutput=False)
    out = nc.declare_dram_parameter("out", [1, NSH], _f32, isOutput=True)

    with tile.TileContext(nc) as tc:
        with (
            tc.tile_pool(name="const", bufs=1) as cpool,
            tc.tile_pool(name="xin", bufs=12) as xpool,
            tc.tile_pool(name="zsq", bufs=2 * HC + 4) as zpool,
            tc.tile_pool(name="zc", bufs=2) as zcpool,
            tc.tile_pool(name="outs", bufs=2) as opool,
            tc.tile_pool(name="gate", bufs=3) as gpool,
            tc.tile_pool(name="pz", bufs=3, space=bass.MemorySpace.PSUM) as pzpool,
            tc.tile_pool(name="pq", bufs=2, space=bass.MemorySpace.PSUM) as pqpool,
        ):
            wc_t = cpool.tile([D, D + 65], _f16, name="wc_t")
            nc.sync.dma_start(out=wc_t[:], in_=wc[:])
            lw_t = wc_t[:, 0:D]
            sel_t = wc_t[:, D:D + 64]
            # Pre-sync the PE on the const DMA with one dummy matmul so real
            # matmuls never stall on it; also starts the HAM warm-up clock.
            warm = pqpool.tile([8, 8], _f32, tag="pq", name="warm")
            nc.tensor.matmul(warm[:], sel_t[:, 0:8], sel_t[:, 0:8])
            # Pre-warm the ACT square table (~2.7us) during the initial DMA.
            aw2 = cpool.tile([D, 1], _f16, name="aw2")
            nc.scalar.activation(
                aw2[:], sel_t[:, 0:1], mybir.ActivationFunctionType.Square
            )

            x_tiles = {}   # chunk -> sbuf tile
            pq_tiles = {}  # chunk -> psum accum tile
            zq_tiles = {}  # group -> sbuf tile

            def load_chunk(c, splits=None):
                # separate sub-tiles per DMA so z matmuls depend only on
                # the slice they read (fast ramp, fine-grained prefetch)
                x_tiles[c] = []
                lo = 0
                for w in (splits or [SUBDMA] * (CHUNK // SUBDMA)):
                    x_q = xpool.tile([D, SUBDMA], _f8, tag="x", name="x_q")
                    nc.sync.dma_start(
                        out=x_q[:, 0:w],
                        in_=xt[:, c * CHUNK + lo:c * CHUNK + lo + w],
                    )
                    x_tiles[c].append((lo, w, x_q))
                    lo += w

            def z_and_square(g):
                c = g // GPC
                if g == 0:
                    # chunk 0 split finer up front so group 0's data lands asap
                    load_chunk(0, splits=[GROUP, SUBDMA - GROUP]
                               + [SUBDMA] * (CHUNK // SUBDMA - 1))
                if g % GPC == 0:
                    pq_tiles[c] = pqpool.tile(
                        [D, SUB], _f32, tag="pq", name="pq"
                    )
                    # Zero the accumulator bank data; all sels accumulate
                    # with start=False onto it.  PSUM has_written bits stay
                    # stale-SET from this bank's previous chunk on exactly
                    # the elements the sels write (the pattern repeats), so
                    # accumulate-onto-zeros is correct and, unlike per-chain
                    # start=True matmuls, order-independent (4 col-group
                    # starts in one bank is a nondeterministic RACE: a
                    # late-scheduled start's zero-region reset wipes other
                    # chains' accumulation).  Alternate engines to split the
                    # ~0.7us cost between the two drain engines.
                    if c % 2 == 0:
                        nc.vector.memset(pq_tiles[c][:], 0.0)
                    else:
                        nc.scalar.mul(pq_tiles[c][:], pq_tiles[c][:], 0.0)
                if g % GPC == HC and c + 1 < NCHUNK:
                    load_chunk(c + 1)  # prefetch ~11us ahead of first use
                pz = pzpool.tile([D, GROUP], _f32, tag="pz", name="pz")
                gbase = (g % GPC) * GROUP
                for s in range(GROUP // SUB):
                    lo = gbase + s * SUB
                    # find the sub-tile holding columns [lo, lo+SUB)
                    for tlo, w, x_q in x_tiles[c]:
                        if tlo <= lo < tlo + w:
                            break
                    nc.tensor.matmul(
                        pz[:, s * SUB:(s + 1) * SUB],
                        lw_t,
                        x_q[:, lo - tlo:lo - tlo + SUB],
                    )
                zq = zpool.tile([D, GROUP], _f16, tag="zq", name="zq")
                dve = (g % GPC) in DVE_PATTERN[c % len(DVE_PATTERN)]
                if dve:
                    # Square on the DVE. DVE reads only ONE operand from
                    # PSUM, so detour: copy pz -> fp16 SBUF (1x), then fp16
                    # tensor_tensor square (2x).
                    zc = zcpool.tile([D, GROUP], _f16, tag="zc", name="zc")
                    nc.vector.tensor_copy(zc[:], pz[:])
                    nc.vector.tensor_mul(zq[:], zc[:], zc[:])
                else:
                    nc.scalar.activation(
                        zq[:], pz[:], mybir.ActivationFunctionType.Square
                    )
                zq_tiles[g] = zq  # consumed one half-chunk later

            def emit_sels(c, subs, gate_zq=None):
                """Colsum matmuls for sub-chunks ``subs`` of chunk c, as one
                batch: the PE pays the L<->sel weight switch only twice.

                gate_zq: if given, the sels read a GpSimd-made copy of the
                selector weights whose FIFO predecessor reads gate_zq — so
                the whole batch becomes schedulable only after gate_zq is
                written.  Without the gate, Tile's priority scheduler
                dribbles the sels into every z-matmul stall in ~3-MM runs,
                paying the L<->sel weight-switch (~200ns) ~85x instead of
                ~32x."""
                if gate_zq is not None:
                    # data-chain so the batch is unschedulable until gate_zq
                    # lands: zq slice -> wc_t col 192, then live copies cols
                    # 128..193 (includes 192) and the sels read live.
                    nc.gpsimd.tensor_copy(wc_t[:, D + 64:D + 65],
                                          gate_zq[:, 0:1])
                    live = gpool.tile([D, 65], _f16, tag="live", name="live")
                    nc.gpsimd.tensor_copy(live[:], wc_t[:, D:D + 65])
                else:
                    live = sel_t
                for s in subs:
                    g = c * GPC + s // 2
                    cg, j = s % 4, s // 4
                    zqq = zq_tiles[g] if s % 2 == 0 else zq_tiles.pop(g)
                    # col-tiled: psum slice base partition 32*cg selects the
                    # array col-group; the 4 cg streams run concurrently.
                    nc.tensor.matmul(
                        pq_tiles[c][32 * cg:32 * cg + 8, :],
                        live[:, 8 * j:8 * j + 8],
                        zqq[:, (s % 2) * SUB:(s % 2 + 1) * SUB],
                        start=False,
                        stop=(s >= SPC - 4),
                        skip_group_check=True,
                        tile_position=(0, 32 * cg),
                    )
                if subs[-1] == SPC - 1:
                    o_t = opool.tile([D, SUB], _f32, tag="o", name="o_t")
                    # +log_den on DVE over the whole [128, 512] accum tile
                    # (only partitions 32*cg + j carry data; rest is junk)
                    nc.vector.tensor_scalar_add(
                        o_t[:], pq_tiles.pop(c)[:], float(log_den)
                    )
                    # sample index within chunk = s*SUB + n, s = 4*j + cg
                    oview = out[:, c * CHUNK:(c + 1) * CHUNK].rearrange(
                        "a (j r) -> (a j) r", j=8
                    )
                    for cg in range(4):
                        nc.sync.dma_start(
                            out=oview[:, cg * SUB:(cg + 1) * SUB],
                            in_=o_t[32 * cg:32 * cg + 8, :],
                        )

            # Software pipeline: z+square stream ahead group by group; the
            # batched sels for each half-chunk run one half-chunk behind,
            # gated so each batch hits the PE as one contiguous burst.
            HALF = list(range(0, 16)), list(range(16, 32))
            for g in range(NG):
                z_and_square(g)
                if g % HC == HC - 1 and g >= 2 * HC - 1:
                    hc = g // HC - 1
                    emit_sels(hc // 2, HALF[hc % 2])
            emit_sels(NCHUNK - 1, HALF[1])  # epilogue, ungated
    nc.compile()
    return nc


def _install_trace_shim():
    """The image lacks ``antenv.axon_hooks``; recreate it and register the
    ctypes NTFF hook that trn_boot would have installed."""
    import types
    import antenv

    if "antenv.axon_hooks" not in sys.modules:
        mod = types.ModuleType("antenv.axon_hooks")
        holder = [None]
        mod.set_axon_ntff_profile_hook = lambda h: holder.__setitem__(0, h)
        mod.get_axon_ntff_profile_hook = lambda: holder[0]
        sys.modules["antenv.axon_hooks"] = mod
        antenv.axon_hooks = mod
    from antenv.axon_hooks import (
        get_axon_ntff_profile_hook,
        set_axon_ntff_profile_hook,
    )

    if get_axon_ntff_profile_hook() is None:
        from trn_agent_boot.trn_boot import _ntff_profile_via_ctypes

        set_axon_ntff_profile_hook(
            _ntff_profile_via_ctypes("/opt/axon/libaxon_pjrt.so")
        )


def kernel(X: np.ndarray, mu: np.ndarray, sigma: np.ndarray, eps: np.ndarray,
           _trace: bool = False) -> np.ndarray:
    global LAST_RESULTS

    # ---- host prep: tiny D x D linear algebra in float64 ----
    sig = (sigma.astype(np.float64) + eps.astype(np.float64))
    S = np.linalg.pinv(sig)
    _, logdet = np.linalg.slogdet(sig)
    log_den = 0.5 * (D * math.log(2.0 * math.pi) + logdet)
    Ssym = 0.5 * (S + S.T)
    L = np.linalg.cholesky(Ssym)          # S = L @ L.T
    Lp = (L / math.sqrt(2.0)).astype(np.float16)  # [d, e] stationary

    wcat = np.zeros((D, D + 65), dtype=np.float16)
    wcat[:, :D] = Lp
    for j in range(8):
        wcat[:, D + 8 * j + j] = 1.0

    # ---- center by mu on host (removes the device-side bias), cast fp8 ----
    f8 = ml_dtypes.float8_e3m4            # TRN FP8_EXP3: max +-15.5
    Xc = np.clip(X - mu[None, :], -15.0, 15.0).astype(f8)
    XT = np.ascontiguousarray(Xc.T)  # [D, N]
    in_maps = []
    for c_id in range(NCORES):
        in_maps.append({
            "xt": np.ascontiguousarray(XT[:, c_id * NSH:(c_id + 1) * NSH]),
            "wc": wcat,
        })

    nc = _build_bass(log_den)
    if _trace:
        _install_trace_shim()
        import tempfile
        import concourse.bass_utils as _bu
        _bu.upload_artifacts = lambda d: "local://" + d  # no S3 in this container
        tmpdir = tempfile.mkdtemp(prefix="bass_trace_")
        print("trace dir:", tmpdir)
        res = run_bass_kernel_spmd(
            nc, in_maps, list(range(NCORES)), trace=True, tmpdir=tmpdir
        )
    else:
        res = run_bass_kernel_spmd(nc, in_maps, list(range(NCORES)))
    LAST_RESULTS = res

    out = np.empty((N, 1), dtype=np.float32)
    for c_id in range(NCORES):
        out[c_id * NSH:(c_id + 1) * NSH, 0] = res.results[c_id]["out"].reshape(-1)
    return out


# revision 24
# speedup vs baseline: 1.0709x; 1.0541x over previous
"""Gaussian NLL loss kernel for Trainium2 (8 NeuronCores, data-parallel).

out[n] = 0.5 * (x_n - mu)^T pinv(sigma+eps) (x_n - mu) + log_den,  shape [N, 1]

Strategy (v4, fp8 e3m4):
  Host: tiny D x D prep (pinv -> symmetrize -> Cholesky L, slogdet); center
  X by mu and cast to fp8 E3M4 (quarters HBM traffic vs fp32; 4 mantissa
  bits keep rel err ~6e-3 << 2e-2 budget); one-time transpose so each
  core's DMA loads are contiguous per partition (features on partitions).
  Device (per core, N/8 samples, software-pipelined):
    z    = (L/sqrt 2)^T x'         mixed fp16(L) x fp8(x) matmul
    zsq  = z^2                     split ~84/44 on ScalarE Square (fused
                                   PSUM-drain+square) and DVE (fp32->fp16
                                   copy, then fp16 2x mul).  The PSUM
                                   drain (ACT@1.2GHz + DVE@0.96GHz per
                                   lane) is the fundamental bottleneck.
    q    = colsum(zsq)             one-hot selector matmuls, 4 CONCURRENT
                                   col-tiled streams (tile_position
                                   32*cg); batched 16 MMs per half-chunk
                                   so the PE pays the stationary-weight
                                   switch (~170ns drain) only 32x total
    out  = q + log_den             one [128,512] DVE add per chunk
  Pure data-parallel: no collectives.
"""

import math
import sys

import numpy as np

sys.path.insert(0, "/opt/trn_rl_repo")

import ml_dtypes

import concourse.bass as bass
import concourse.bacc as bacc
import concourse.mybir as mybir
import concourse.tile as tile
from concourse.bass_utils import run_bass_kernel_spmd

N, D = 1048576, 128
NCORES = 8
NSH = N // NCORES   # 131072 samples per core
CHUNK = 16384       # samples per x tile (2MB fp8), loaded as 4 sub-DMAs
SUBDMA = 4096       # columns per input DMA (512KB)
GROUP = 1024        # samples per square op (2 PSUM banks)
SUB = 512           # samples per matmul (PSUM bank limit)
GPC = CHUNK // GROUP   # 16 groups per chunk
SPC = CHUNK // SUB     # 32 colsum sub-chunks per chunk
HC = GPC // 2          # groups per half-chunk (sel batch unit)
NCHUNK = NSH // CHUNK  # 8 chunks per core
NG = NSH // GROUP      # 128 groups per core
# which groups in a chunk square on DVE (rest on ScalarE); ACT ~1103ns vs
# DVE copy+mul ~1912ns per group -> 83:45 split over the 128 groups.
DVE_5 = (1, 4, 7, 10, 13)
DVE_6 = (1, 4, 6, 9, 12, 15)
DVE_PATTERN = (DVE_6, DVE_6, DVE_5, DVE_6, DVE_6, DVE_5, DVE_6, DVE_5)

_f32 = mybir.dt.float32
_f16 = mybir.dt.float16
_f8 = mybir.dt.float8e3

LAST_RESULTS = None  # BassKernelResults of the most recent run (for test.py)


def _build_bass(log_den: float) -> bass.Bass:
    nc = bacc.Bacc()
    xt = nc.declare_dram_parameter("xt", [D, NSH], _f8, isOutput=False)
    # fp16 consts: cols 0-127 = L/sqrt2; cols 128+8j..+8 = one-hot sel mat
    # j; col 192 = scratch the sel-batch gate writes into
    wc = nc.declare_dram_parameter("wc", [D, D + 65], _f16, isOutput=False)
    out = nc.declare_dram_parameter("out", [1, NSH], _f32, isOutput=True)

    with tile.TileContext(nc) as tc:
        with (
            tc.tile_pool(name="const", bufs=1) as cpool,
            tc.tile_pool(name="xin", bufs=12) as xpool,
            tc.tile_pool(name="zsq", bufs=2 * HC + 4) as zpool,
            tc.tile_pool(name="zc", bufs=2) as zcpool,
            tc.tile_pool(name="outs", bufs=2) as opool,
            tc.tile_pool(name="gate", bufs=3) as gpool,
            tc.tile_pool(name="pz", bufs=3, space=bass.MemorySpace.PSUM) as pzpool,
            tc.tile_pool(name="pq", bufs=2, space=bass.MemorySpace.PSUM) as pqpool,
        ):
            wc_t = cpool.tile([D, D + 65], _f16, name="wc_t")
            nc.sync.dma_start(out=wc_t[:], in_=wc[:])
            lw_t = wc_t[:, 0:D]
            sel_t = wc_t[:, D:D + 64]
            # Pre-sync the PE on the const DMA with one dummy matmul so real
            # matmuls never stall on it; also starts the HAM warm-up clock.
            warm = pqpool.tile([8, 8], _f32, tag="pq", name="warm")
            nc.tensor.matmul(warm[:], sel_t[:, 0:8], sel_t[:, 0:8])
            # Pre-warm the ACT square table (~2.7us) during the initial DMA.
            aw2 = cpool.tile([D, 1], _f16, name="aw2")
            nc.scalar.activation(
                aw2[:], sel_t[:, 0:1], mybir.ActivationFunctionType.Square
            )

            x_tiles = {}   # chunk -> sbuf tile
            pq_tiles = {}  # chunk -> psum accum tile
            zq_tiles = {}  # group -> sbuf tile

            def load_chunk(c, splits=None):
                # separate sub-tiles per DMA so z matmuls depend only on
                # the slice they read (fast ramp, fine-grained prefetch)
                x_tiles[c] = []
                lo = 0
                for w in (splits or [SUBDMA] * (CHUNK // SUBDMA)):
                    x_q = xpool.tile([D, SUBDMA], _f8, tag="x", name="x_q")
                    nc.sync.dma_start(
                        out=x_q[:, 0:w],
                        in_=xt[:, c * CHUNK + lo:c * CHUNK + lo + w],
                    )
                    x_tiles[c].append((lo, w, x_q))
                    lo += w

            def z_and_square(g):
                c = g // GPC
                if g == 0:
                    # chunk 0 split finer up front so group 0's data lands asap
                    load_chunk(0, splits=[GROUP, SUBDMA - GROUP]
                               + [SUBDMA] * (CHUNK // SUBDMA - 1))
                if g % GPC == 0:
                    pq_tiles[c] = pqpool.tile(
                        [D, SUB], _f32, tag="pq", name="pq"
                    )
                    # Zero the accumulator bank data; all sels accumulate
                    # with start=False onto it.  PSUM has_written bits stay
                    # stale-SET from this bank's previous chunk on exactly
                    # the elements the sels write (the pattern repeats), so
                    # accumulate-onto-zeros is correct and, unlike per-chain
                    # start=True matmuls, order-independent (4 col-group
                    # starts in one bank is a nondeterministic RACE: a
                    # late-scheduled start's zero-region reset wipes other
                    # chains' accumulation).  Alternate engines to split the
                    # ~0.7us cost between the two drain engines.
                    if c % 2 == 0:
                        nc.vector.memset(pq_tiles[c][:], 0.0)
                    else:
                        # scale-by-0 of a KNOWN-FINITE source (the clipped
                        # fp8 x tile): reading the stale psum itself here is
                        # a latent NaN bomb (0 * inf stale bits = NaN).
                        nc.scalar.mul(pq_tiles[c][:],
                                      x_tiles[c][0][2][:, 0:SUB], 0.0)
                if g % GPC == HC and c + 1 < NCHUNK:
                    load_chunk(c + 1)  # prefetch ~11us ahead of first use
                pz = pzpool.tile([D, GROUP], _f32, tag="pz", name="pz")
                gbase = (g % GPC) * GROUP
                for s in range(GROUP // SUB):
                    lo = gbase + s * SUB
                    # find the sub-tile holding columns [lo, lo+SUB)
                    for tlo, w, x_q in x_tiles[c]:
                        if tlo <= lo < tlo + w:
                            break
                    nc.tensor.matmul(
                        pz[:, s * SUB:(s + 1) * SUB],
                        lw_t,
                        x_q[:, lo - tlo:lo - tlo + SUB],
                    )
                zq = zpool.tile([D, GROUP], _f16, tag="zq", name="zq")
                dve = (g % GPC) in DVE_PATTERN[c % len(DVE_PATTERN)]
                if dve:
                    # Square on the DVE. DVE reads only ONE operand from
                    # PSUM, so detour: copy pz -> fp16 SBUF (1x), then fp16
                    # tensor_tensor square (2x).
                    zc = zcpool.tile([D, GROUP], _f16, tag="zc", name="zc")
                    nc.vector.tensor_copy(zc[:], pz[:])
                    nc.vector.tensor_mul(zq[:], zc[:], zc[:])
                else:
                    nc.scalar.activation(
                        zq[:], pz[:], mybir.ActivationFunctionType.Square
                    )
                zq_tiles[g] = zq  # consumed one half-chunk later

            def emit_sels(c, subs, gate_zq=None):
                """Colsum matmuls for sub-chunks ``subs`` of chunk c, as one
                batch: the PE pays the L<->sel weight switch only twice.

                gate_zq: if given, the sels read a GpSimd-made copy of the
                selector weights whose FIFO predecessor reads gate_zq — so
                the whole batch becomes schedulable only after gate_zq is
                written.  Without the gate, Tile's priority scheduler
                dribbles the sels into every z-matmul stall in ~3-MM runs,
                paying the L<->sel weight-switch (~200ns) ~85x instead of
                ~32x."""
                if gate_zq is not None:
                    # data-chain so the batch is unschedulable until gate_zq
                    # lands: zq slice -> wc_t col 192, then live copies cols
                    # 128..193 (includes 192) and the sels read live.
                    nc.gpsimd.tensor_copy(wc_t[:, D + 64:D + 65],
                                          gate_zq[:, 0:1])
                    live = gpool.tile([D, 65], _f16, tag="live", name="live")
                    nc.gpsimd.tensor_copy(live[:], wc_t[:, D:D + 65])
                else:
                    live = sel_t
                for s in subs:
                    g = c * GPC + s // 2
                    cg, j = s % 4, s // 4
                    zqq = zq_tiles[g] if s % 2 == 0 else zq_tiles.pop(g)
                    # col-tiled: psum slice base partition 32*cg selects the
                    # array col-group; the 4 cg streams run concurrently.
                    nc.tensor.matmul(
                        pq_tiles[c][32 * cg:32 * cg + 8, :],
                        live[:, 8 * j:8 * j + 8],
                        zqq[:, (s % 2) * SUB:(s % 2 + 1) * SUB],
                        start=False,
                        stop=(s >= SPC - 4),
                        skip_group_check=True,
                        tile_position=(0, 32 * cg),
                    )
                if subs[-1] == SPC - 1:
                    o_t = opool.tile([D, SUB], _f32, tag="o", name="o_t")
                    # +log_den on DVE over the whole [128, 512] accum tile
                    # (only partitions 32*cg + j carry data; rest is junk)
                    nc.vector.tensor_scalar_add(
                        o_t[:], pq_tiles.pop(c)[:], float(log_den)
                    )
                    # sample index within chunk = s*SUB + n, s = 4*j + cg
                    oview = out[:, c * CHUNK:(c + 1) * CHUNK].rearrange(
                        "a (j r) -> (a j) r", j=8
                    )
                    for cg in range(4):
                        nc.sync.dma_start(
                            out=oview[:, cg * SUB:(cg + 1) * SUB],
                            in_=o_t[32 * cg:32 * cg + 8, :],
                        )

            # Software pipeline: z+square stream ahead group by group; the
            # batched sels for each half-chunk run one half-chunk behind,
            # gated so each batch hits the PE as one contiguous burst.
            HALF = list(range(0, 16)), list(range(16, 32))
            for g in range(NG):
                z_and_square(g)
                if g % HC == HC - 1 and g >= 2 * HC - 1:
                    hc = g // HC - 1
                    emit_sels(hc // 2, HALF[hc % 2])
            emit_sels(NCHUNK - 1, HALF[1])  # epilogue, ungated
    nc.compile()
    return nc


def _install_trace_shim():
    """The image lacks ``antenv.axon_hooks``; recreate it and register the
    ctypes NTFF hook that trn_boot would have installed."""
    import types
    import antenv

    if "antenv.axon_hooks" not in sys.modules:
        mod = types.ModuleType("antenv.axon_hooks")
        holder = [None]
        mod.set_axon_ntff_profile_hook = lambda h: holder.__setitem__(0, h)
        mod.get_axon_ntff_profile_hook = lambda: holder[0]
        sys.modules["antenv.axon_hooks"] = mod
        antenv.axon_hooks = mod
    from antenv.axon_hooks import (
        get_axon_ntff_profile_hook,
        set_axon_ntff_profile_hook,
    )

    if get_axon_ntff_profile_hook() is None:
        from trn_agent_boot.trn_boot import _ntff_profile_via_ctypes

        set_axon_ntff_profile_hook(
            _ntff_profile_via_ctypes("/opt/axon/libaxon_pjrt.so")
        )


def kernel(X: np.ndarray, mu: np.ndarray, sigma: np.ndarray, eps: np.ndarray,
           _trace: bool = False) -> np.ndarray:
    global LAST_RESULTS

    # ---- host prep: tiny D x D linear algebra in float64 ----
    sig = (sigma.astype(np.float64) + eps.astype(np.float64))
    S = np.linalg.pinv(sig)
    _, logdet = np.linalg.slogdet(sig)
    log_den = 0.5 * (D * math.log(2.0 * math.pi) + logdet)
    Ssym = 0.5 * (S + S.T)
    L = np.linalg.cholesky(Ssym)          # S = L @ L.T
    Lp = (L / math.sqrt(2.0)).astype(np.float16)  # [d, e] stationary

    wcat = np.zeros((D, D + 65), dtype=np.float16)
    wcat[:, :D] = Lp
    for j in range(8):
        wcat[:, D + 8 * j + j] = 1.0

    # ---- center by mu on host (removes the device-side bias), cast fp8 ----
    f8 = ml_dtypes.float8_e3m4            # TRN FP8_EXP3: max +-15.5
    Xc = np.clip(X - mu[None, :], -15.0, 15.0).astype(f8)
    XT = np.ascontiguousarray(Xc.T)  # [D, N]
    in_maps = []
    for c_id in range(NCORES):
        in_maps.append({
            "xt": np.ascontiguousarray(XT[:, c_id * NSH:(c_id + 1) * NSH]),
            "wc": wcat,
        })

    nc = _build_bass(log_den)
    if _trace:
        _install_trace_shim()
        import tempfile
        import concourse.bass_utils as _bu
        _bu.upload_artifacts = lambda d: "local://" + d  # no S3 in this container
        tmpdir = tempfile.mkdtemp(prefix="bass_trace_")
        print("trace dir:", tmpdir)
        res = run_bass_kernel_spmd(
            nc, in_maps, list(range(NCORES)), trace=True, tmpdir=tmpdir
        )
    else:
        res = run_bass_kernel_spmd(nc, in_maps, list(range(NCORES)))
    LAST_RESULTS = res

    out = np.empty((N, 1), dtype=np.float32)
    for c_id in range(NCORES):
        out[c_id * NSH:(c_id + 1) * NSH, 0] = res.results[c_id]["out"].reshape(-1)
    return out
